# revision 20
# baseline (speedup 1.0000x reference)
"""nn_GroupAttention Trainium2 kernel (8-core SPMD).

Sharding: core = (b, s): b = core//2 (batch), s = core%2 (head half).
Each core handles batch b, heads 8s..8s+7 = d_model channels 512s..512s+512
(= groups 2s, 2s+1).  Device tensors are channel-major ("transposed") so all
matmul contractions run over the partition dim.  The transformer-XL rel-shift
is a diagonal-access-pattern SBUF->SBUF DMA; the causal band mask comes free
from skewing a constant -1e5 pad region.  The inter (Winter) projection
contraction over all 1024 channels finishes with a pair-wise AllReduce.

All core-dependence lives in host-prepared input slices (SPMD program is
identical).  The w-side channels are permuted so the core's own two groups
occupy channel tiles 0..3; WiqT rows are permuted to match.
"""
import sys

sys.path.insert(0, "/opt/trn_rl_repo")
import numpy as np

Q, M, KLEN, B = 512, 512, 1024, 4
D, H, DH, G = 1024, 16, 64, 4
DG = D // G
EPS = 1e-6
SCALE = 0.125
VMASK = -1e5
SW = 1664          # skew scratch width (>= 127 + 1536)
JR_PAD = 1536      # staging width: 1024 jr cols + 512 mask cols
VALID = {it: [jt for jt in range(8) if jt - it <= 4] for it in range(4)}

_cache = {}


def _build_nc():
    import concourse.bass as bass
    import concourse.bacc as bacc
    import concourse.mybir as mybir
    import concourse.tile as tile

    BF = mybir.dt.bfloat16
    F32 = mybir.dt.float32
    AF = mybir.ActivationFunctionType
    ALU = mybir.AluOpType

    nc = bacc.Bacc("TRN2", target_bir_lowering=False, debug=False,
                   num_devices=8)

    def din(name, shape, dt=BF):
        return nc.declare_dram_parameter(name, list(shape), dt, isOutput=False)

    kvT = din("kvT", [D, KLEN])
    wTp = din("wTp", [D, Q])              # channel-PERMUTED w^T
    wres = din("wres", [512, Q], mybir.dt.float32)
    rT = din("rT", [512, KLEN])
    WkT = din("WkT", [D, 512])
    WvT = din("WvT", [D, 512])
    WiqT = din("WiqT", [D, DG])           # row-permuted to match wTp
    WqT = din("WqT", [2, DG, DG])
    WrT = din("WrT", [2, DG, DG])
    WintraT = din("WintraT", [2, DG, DG])
    WinterT = din("WinterT", [512, DG])
    kbeta = din("kbeta", [128, 4], F32)
    rwb = din("rwb", [128, 4], F32)
    rrb = din("rrb", [128, 4], F32)
    ident = din("ident", [128, 128])
    out = nc.declare_dram_parameter("out", [512, Q], F32, isOutput=True)

    cc_in = nc.dram_tensor("cc_in", [DG, Q], F32)
    cc_out = nc.dram_tensor("cc_out", [DG, Q], F32)

    with tile.TileContext(nc) as tc:
        with (
            tc.tile_pool(name="persist", bufs=1) as P,
            tc.tile_pool(name="work", bufs=2) as W,
            tc.tile_pool(name="epool", bufs=2) as EP,
        ):
            ones_col = P.tile([128, 1], BF, tag="ones")
            nc.gpsimd.memset(ones_col[:], 1.0)
            ones_row = P.tile([1, 128], BF, tag="onesrow")
            nc.gpsimd.memset(ones_row[:], 1.0)
            id_sb = P.tile([128, 128], BF, tag="id")
            nc.sync.dma_start(id_sb[:], ident[:])
            id1f = P.tile([1, 1], F32, tag="id1f")
            nc.gpsimd.memset(id1f[:], 1.0)

            KT_sb = P.tile([128, 4 * KLEN], BF, tag="KT")       # dt-major
            V_sb = P.tile([128, 8 * 520], BF, tag="V")          # jt x (h x 65)
            QbT = P.tile([128, 4 * Q], BF, tag="QbT")
            QcT = P.tile([128, 4 * Q], BF, tag="QcT")
            rhT = P.tile([128, 4 * KLEN], BF, tag="rhT")        # dt-major
            attn_sb = P.tile([128, 4 * 512], BF, tag="attn")    # it-major

            staging = [P.tile([128, JR_PAD], BF, tag=f"stg{i}", name=f"stg{i}") for i in range(4)]
            scratch = [P.tile([128, SW], BF, tag=f"scr{i}", name=f"scr{i}") for i in range(8)]
            for st in staging:
                nc.gpsimd.memset(st[:, KLEN:JR_PAD], VMASK)

            # ================= phase A: layernorm stats =================
            SPAN = tc.tile_pool(name="span_ac", bufs=1)
            SPANc = SPAN.__enter__()
            xhat = SPANc.tile([128, 8 * KLEN], BF, tag="xhat")
            xwhat = SPANc.tile([128, 8 * Q], BF, tag="xwhat")
            with (
                tc.tile_pool(name="lnA", bufs=1) as LA,
                tc.tile_pool(name="lnAw", bufs=2) as LAW,
            ):
                rho_bt = LA.tile([128, KLEN], BF, tag="rhobt")
                mrh_bt = LA.tile([128, KLEN], BF, tag="mrhbt")
                rhow_bt = LA.tile([128, 4 * Q], BF, tag="rhowbt")
                mrhw_bt = LA.tile([128, 4 * Q], BF, tag="mrhwbt")
                wT_sb = LA.tile([128, 8 * Q], BF, tag="wraw")
                nc.sync.dma_start(
                    wT_sb[:].rearrange("p (ct q) -> p ct q", ct=8),
                    wTp[:].rearrange("(ct p) q -> p ct q", p=128))

                PSAx = tc.tile_pool(name="psA", bufs=1, space="PSUM")
                PSA = PSAx.__enter__()
                pscol = PSA.tile([128, 48], F32, tag="pscol")
                ps_sum = PSA.tile([1, KLEN], F32, tag="pssum")
                ps_sq = PSA.tile([1, KLEN], F32, tag="pssq")
                for ct in range(8):
                    kvc = LAW.tile([128, KLEN], BF, tag="kvc")
                    nc.sync.dma_start(kvc[:], kvT[ct * 128:(ct + 1) * 128, :])
                    sq = LAW.tile([128, KLEN], BF, tag="sq")
                    nc.scalar.activation(sq[:], kvc[:], AF.Square)
                    for nb in range(2):
                        sl = slice(nb * 512, (nb + 1) * 512)
                        nc.tensor.matmul(
                            ps_sum[:, sl], ones_col[:], kvc[:, sl],
                            start=(ct == 0), stop=(ct == 7))
                        nc.tensor.matmul(ps_sq[:, sl], ones_col[:], sq[:, sl],
                                         start=(ct == 0), stop=(ct == 7))
                for k in range(8):
                    for j, src_t in ((0, ps_sum), (8, ps_sq)):
                        rowc = LAW.tile([1, 128], F32, tag="rowc", bufs=4,
                                        name=f"rowc{j}{k}")
                        nc.scalar.activation(rowc[:], src_t[:, k * 128:(k + 1) * 128],
                                             AF.Copy)
                        nc.tensor.matmul(pscol[:, j + k:j + k + 1], rowc[:], id1f[:],
                                         start=True, stop=True)
                for g in range(4):
                    ps_gs = PSA.tile([1, Q], F32, tag="psg", bufs=2, name=f"psgs{g}")
                    ps_gq = PSA.tile([1, Q], F32, tag="psg", bufs=2, name=f"psgq{g}")
                    for kt in range(2):
                        ct = 2 * g + kt
                        sqw = LAW.tile([128, Q], BF, tag="sqw")
                        nc.scalar.activation(sqw[:], wT_sb[:, ct * Q:(ct + 1) * Q],
                                             AF.Square)
                        nc.tensor.matmul(ps_gs[:], ones_col[:],
                                         wT_sb[:, ct * Q:(ct + 1) * Q],
                                         start=(kt == 0), stop=(kt == 1))
                        nc.tensor.matmul(ps_gq[:], ones_col[:], sqw[:],
                                         start=(kt == 0), stop=(kt == 1))
                    for it in range(4):
                        for j, src_t in ((16, ps_gs), (32, ps_gq)):
                            k = j + g * 4 + it
                            rowc = LAW.tile([1, 128], F32, tag="rowc", bufs=4,
                                            name=f"rowcg{k}")
                            nc.scalar.activation(
                                rowc[:], src_t[:, it * 128:(it + 1) * 128], AF.Copy)
                            nc.tensor.matmul(pscol[:, k:k + 1], rowc[:], id1f[:],
                                             start=True, stop=True)
                stc = LA.tile([128, 48], F32, tag="stc")
                nc.vector.tensor_copy(stc[:], pscol[:])
                PSAx.__exit__(None, None, None)
                # stc: 0..7 kv-sum | 8..15 kv-sumsq | 16..31 w-sum | 32..47 w-sumsq

                def stats_math(sum_ap, sq_ap, n, rho_out, mrh_out, width):
                    mu = LA.tile([128, width], F32, tag=f"mu{n}{width}")
                    va = LA.tile([128, width], F32, tag=f"va{n}{width}")
                    tm = LA.tile([128, width], F32, tag=f"tm{n}{width}")
                    nc.vector.tensor_scalar_mul(mu[:], sum_ap, 1.0 / n)
                    nc.vector.tensor_mul(tm[:], mu[:], mu[:])
                    nc.vector.tensor_scalar_mul(va[:], sq_ap, 1.0 / (n - 1))
                    nc.vector.scalar_tensor_tensor(
                        va[:], tm[:], -float(n) / (n - 1), va[:],
                        op0=ALU.mult, op1=ALU.add)
                    nc.scalar.activation(va[:], va[:], AF.Sqrt)
                    nc.vector.tensor_scalar_add(va[:], va[:], EPS)
                    nc.vector.reciprocal(va[:], va[:])
                    nc.vector.tensor_copy(rho_out, va[:])     # -> bf16
                    nc.vector.tensor_mul(tm[:], mu[:], va[:])
                    nc.vector.tensor_copy(mrh_out, tm[:])     # -> bf16

                pk = LA.tile([128, 48], BF, tag="pk")
                stats_math(stc[:, 0:8], stc[:, 8:16], D, pk[:, 0:8], pk[:, 8:16], 8)
                stats_math(stc[:, 16:32], stc[:, 32:48], DG, pk[:, 16:32], pk[:, 32:48], 16)

                # broadcast tiles: bt[:, c*128+p] = pk[p, k].  One matmul per
                # (stat, chunk): lhsT = pk column free-broadcast, rhs = identity
                # -> out[m, n] = pk[n, k].
                with tc.tile_pool(name="psA2", bufs=2, space="PSUM") as PSA2:
                    bjobs = [(rho_bt, 0, 0, 8), (mrh_bt, 0, 8, 8),
                             (rhow_bt, 0, 16, 8), (rhow_bt, 1024, 24, 8),
                             (mrhw_bt, 0, 32, 8), (mrhw_bt, 1024, 40, 8)]
                    for bi, (dst, doff, rbase, nch) in enumerate(bjobs):
                        bc = PSA2.tile([128, 1024], F32, tag="bcps", name=f"bc{bi}")
                        for c in range(nch):
                            col = pk[:, rbase + c:rbase + c + 1].broadcast_to([128, 128])
                            nc.tensor.matmul(bc[:, c * 128:(c + 1) * 128],
                                             col, id_sb[:], start=True, stop=True)
                        nc.vector.tensor_copy(dst[:, doff:doff + 1024], bc[:])

                for ct in range(8):
                    sl = slice(ct * KLEN, (ct + 1) * KLEN)
                    kvc = LAW.tile([128, KLEN], BF, tag="kvc")
                    nc.sync.dma_start(kvc[:], kvT[ct * 128:(ct + 1) * 128, :])
                    t = LAW.tile([128, KLEN], BF, tag="nrm")
                    nc.vector.tensor_mul(t[:], kvc[:], rho_bt[:])
                    nc.vector.tensor_sub(xhat[:, sl], t[:], mrh_bt[:])
                for ct in range(8):
                    g = ct // 2
                    sl = slice(ct * Q, (ct + 1) * Q)
                    gsl = slice(g * Q, (g + 1) * Q)
                    t = LAW.tile([128, Q], BF, tag="nrmw")
                    nc.vector.tensor_mul(t[:], wT_sb[:, sl], rhow_bt[:, gsl])
                    nc.vector.tensor_sub(xwhat[:, sl], t[:], mrhw_bt[:, gsl])

            # ================= phase C: projections =================
            with (
                tc.tile_pool(name="projw", bufs=2) as PW,
                tc.tile_pool(name="projcw", bufs=1) as PC,
                tc.tile_pool(name="psP", bufs=2, space="PSUM") as PSP,
                tc.tile_pool(name="psP2", bufs=2, space="PSUM") as PSP2,
            ):
                Wiq_sb = PC.tile([128, 8 * DG], BF, tag="wiq")
                Wq_sb = PC.tile([128, 4 * DG], BF, tag="wq")
                Wr_sb = PC.tile([128, 4 * DG], BF, tag="wr")
                Wk_sb = PC.tile([128, 8 * 512], BF, tag="wk")
                Wv_sb = PC.tile([128, 8 * 512], BF, tag="wv")
                kbeta_sb = PC.tile([128, 4], F32, tag="kbeta")
                rwb_sb = PC.tile([128, 4], F32, tag="rwb")
                rrb_sb = PC.tile([128, 4], F32, tag="rrb")
                nc.sync.dma_start(kbeta_sb[:], kbeta[:])
                nc.sync.dma_start(rwb_sb[:], rwb[:])
                nc.sync.dma_start(rrb_sb[:], rrb[:])
                nc.sync.dma_start(
                    Wiq_sb[:].rearrange("p (kt o) -> p kt o", kt=8),
                    WiqT[:].rearrange("(kt p) o -> p kt o", p=128))
                nc.sync.dma_start(
                    Wk_sb[:].rearrange("p (kt o) -> p kt o", kt=8),
                    WkT[:].rearrange("(kt p) o -> p kt o", p=128))
                nc.sync.dma_start(
                    Wv_sb[:].rearrange("p (kt o) -> p kt o", kt=8),
                    WvT[:].rearrange("(kt p) o -> p kt o", p=128))
                nc.sync.dma_start(
                    Wq_sb[:].rearrange("p (b o) -> p b o", b=4),
                    WqT[:].rearrange("g (kt p) o -> p (g kt) o", p=128))
                nc.sync.dma_start(
                    Wr_sb[:].rearrange("p (b o) -> p b o", b=4),
                    WrT[:].rearrange("g (kt p) o -> p (g kt) o", p=128))
                for dt in range(4):
                    ps_k = PSP.tile([128, KLEN], F32, tag="pskr", name=f"psk{dt}")
                    for kt in range(8):
                        for nb in range(2):
                            nc.tensor.matmul(
                                ps_k[:, nb * 512:(nb + 1) * 512],
                                Wk_sb[:, kt * 512 + dt * 128:kt * 512 + (dt + 1) * 128],
                                xhat[:, kt * KLEN + nb * 512:kt * KLEN + (nb + 1) * 512],
                                start=(kt == 0), stop=(kt == 7))
                    nc.vector.tensor_scalar_add(
                        KT_sb[:, dt * KLEN:(dt + 1) * KLEN], ps_k[:],
                        kbeta_sb[:, dt:dt + 1])
                for jt in range(8):
                    ps_v = PSP2.tile([128, 512], F32, tag="ps512", name=f"psv{jt}")
                    for kt in range(8):
                        nc.tensor.matmul(
                            ps_v[:],
                            xhat[:, kt * KLEN + jt * 128:kt * KLEN + (jt + 1) * 128],
                            Wv_sb[:, kt * 512:(kt + 1) * 512],
                            start=(kt == 0), stop=(kt == 7))
                    import concourse.bass as _bassmod
                    vdst = _bassmod.AP(V_sb[:].tensor, V_sb[:].offset + jt * 520,
                                       [[V_sb[:].ap[0][0], 128], [65, 8], [1, 64]])
                    nc.vector.tensor_copy(vdst, ps_v[:].rearrange("p (h c) -> p h c", h=8))
                    nc.gpsimd.memset(
                        _bassmod.AP(V_sb[:].tensor, V_sb[:].offset + jt * 520 + 64,
                                    [[V_sb[:].ap[0][0], 128], [65, 8], [1, 1]]), 1.0)

                ps_qg = PSP2.tile([128, 2 * Q], F32, tag="psqg", bufs=1)
                for mt in range(2):
                    for kt in range(8):
                        nc.tensor.matmul(
                            ps_qg[:, mt * Q:(mt + 1) * Q],
                            Wiq_sb[:, kt * DG + mt * 128:kt * DG + (mt + 1) * 128],
                            xwhat[:, kt * Q:(kt + 1) * Q],
                            start=(kt == 0), stop=(kt == 7))
                qg_sb = PW.tile([128, 2 * Q], BF, tag="qg")
                nc.vector.tensor_copy(qg_sb[:], ps_qg[:])

                for dt in range(4):
                    gl, mt = dt // 2, dt % 2
                    ps_qi = PSP2.tile([128, Q], F32, tag="ps512", name=f"psqi{dt}")
                    for kt in range(2):
                        blk = gl * 2 + kt
                        nc.tensor.matmul(
                            ps_qi[:],
                            Wq_sb[:, blk * DG + mt * 128:blk * DG + (mt + 1) * 128],
                            xwhat[:, blk * Q:(blk + 1) * Q],
                            start=(kt == 0), stop=(kt == 1))
                    nc.vector.scalar_tensor_tensor(
                        QbT[:, dt * Q:(dt + 1) * Q], ps_qi[:], rwb_sb[:, dt:dt + 1],
                        qg_sb[:, mt * Q:(mt + 1) * Q], op0=ALU.add, op1=ALU.add)
                    nc.vector.scalar_tensor_tensor(
                        QcT[:, dt * Q:(dt + 1) * Q], ps_qi[:], rrb_sb[:, dt:dt + 1],
                        qg_sb[:, mt * Q:(mt + 1) * Q], op0=ALU.add, op1=ALU.add)

                rT_sb = PW.tile([128, 4 * KLEN], BF, tag="rtin")
                nc.sync.dma_start(
                    rT_sb[:].rearrange("p (ct j) -> p ct j", ct=4),
                    rT[:].rearrange("(ct p) j -> p ct j", p=128))
                for dt in range(4):
                    gl, mt = dt // 2, dt % 2
                    ps_r = PSP.tile([128, KLEN], F32, tag="pskr", name=f"psr{dt}")
                    for kt in range(2):
                        blk = gl * 2 + kt
                        for nb in range(2):
                            nc.tensor.matmul(
                                ps_r[:, nb * 512:(nb + 1) * 512],
                                Wr_sb[:, blk * DG + mt * 128:blk * DG + (mt + 1) * 128],
                                rT_sb[:, blk * KLEN + nb * 512:blk * KLEN + (nb + 1) * 512],
                                start=(kt == 0), stop=(kt == 1))
                    nc.vector.tensor_copy(rhT[:, dt * KLEN:(dt + 1) * KLEN], ps_r[:])

            SPAN.__exit__(None, None, None)

            # ================= phase D: attention =================
            with (
                tc.tile_pool(name="psS", bufs=2, space="PSUM") as PSS,
                tc.tile_pool(name="psBD", bufs=2, space="PSUM") as PSBD,
                tc.tile_pool(name="psAV", bufs=2, space="PSUM") as PSAV,
            ):
                for h in range(8):
                    dt, off = h // 2, (h % 2) * 64
                    par = 0
                    qrow = slice(off, off + 64)
                    E_sb = EP.tile([128, 8 * 512], BF, tag="E")
                    for it in range(4):
                        jr0 = 384 - it * 128
                        ps_bd = PSBD.tile([128, KLEN], F32, tag="psbd")
                        segs = [(jr0, 512), (512, 1024)]
                        for lo, hi in segs:
                            nc.tensor.matmul(
                                ps_bd[:, lo:hi],
                                QcT[qrow, dt * Q + it * 128:dt * Q + (it + 1) * 128],
                                rhT[qrow, dt * KLEN + lo:dt * KLEN + hi],
                                start=True, stop=True)
                        stg = staging[par + it]
                        nc.vector.tensor_copy(stg[:, jr0:KLEN], ps_bd[:, jr0:KLEN])
                        scr = scratch[(h % 2) * 4 + it]
                        diag = bass.AP(scr[:].tensor, jr0,
                                       [[SW + 1, 128], [1, JR_PAD - jr0]])
                        nc.sync.dma_start(out=diag, in_=stg[:, jr0:JR_PAD])
                    for jt in range(8):
                        i0 = max(0, jt - 4) * 128
                        ps_s = PSS.tile([128, 512], F32, tag="pss")
                        nc.tensor.matmul(
                            ps_s[:, i0:512],
                            KT_sb[qrow, dt * KLEN + jt * 128:dt * KLEN + (jt + 1) * 128],
                            QbT[qrow, dt * Q + i0:dt * Q + 512],
                            start=True, stop=False, skip_group_check=True)
                        for ib in range(max(0, jt - 4), 4):
                            col0 = jt * 128 + 511 - ib * 128
                            nc.tensor.matmul(
                                ps_s[:, ib * 128:(ib + 1) * 128],
                                scratch[(h % 2) * 4 + ib][:, col0:col0 + 128],
                                id_sb[:],
                                start=False, stop=(ib == 3), skip_group_check=True)
                        nc.scalar.activation(E_sb[:, jt * 512 + i0:(jt + 1) * 512],
                                             ps_s[:, i0:512], AF.Exp, scale=SCALE)
                    for it in range(4):
                        ps_av = PSAV.tile([128, 65], F32, tag="psav")
                        jts = VALID[it]
                        for idx, jt in enumerate(jts):
                            lhs = E_sb[:, jt * 512 + it * 128:jt * 512 + (it + 1) * 128]
                            nc.tensor.matmul(ps_av[:], lhs,
                                             V_sb[:, jt * 520 + h * 65:jt * 520 + (h + 1) * 65],
                                             start=(idx == 0), stop=(idx == len(jts) - 1))
                        rec = W.tile([128, 1], F32, tag="rec")
                        nc.vector.reciprocal(rec[:], ps_av[:, 64:65])
                        nc.vector.tensor_scalar_mul(
                            attn_sb[:, it * 512 + h * 64:it * 512 + (h + 1) * 64],
                            ps_av[:, 0:64], rec[:])

            # ================= phase E: output projection =================
            with (
                tc.tile_pool(name="phE", bufs=1) as PE_,
                tc.tile_pool(name="psT", bufs=2, space="PSUM") as PST,
                tc.tile_pool(name="psI", bufs=1, space="PSUM") as PSI,
                tc.tile_pool(name="psJ", bufs=2, space="PSUM") as PSJ,
            ):
                avT_sb = PE_.tile([128, 4 * 512], BF, tag="avT")
                wres_sb = PE_.tile([128, 4 * 512], F32, tag="wres")
                inter_sb = PE_.tile([128, 2 * 512], F32, tag="inter")
                inter_rd = PE_.tile([128, 2 * 512], F32, tag="interrd")
                Wintra_sb = PE_.tile([128, 4 * DG], BF, tag="wintra")
                Winter_sb = PE_.tile([128, 4 * DG], BF, tag="winter")
                nc.sync.dma_start(
                    wres_sb[:].rearrange("p (t q) -> p t q", t=4),
                    wres[:].rearrange("(t p) q -> p t q", p=128))
                nc.sync.dma_start(
                    Wintra_sb[:].rearrange("p (b o) -> p b o", b=4),
                    WintraT[:].rearrange("g (kt p) o -> p (g kt) o", p=128))
                nc.sync.dma_start(
                    Winter_sb[:].rearrange("p (kt o) -> p kt o", kt=4),
                    WinterT[:].rearrange("(kt p) o -> p kt o", p=128))
                for dt in range(4):
                    ps_t = PST.tile([128, 512], F32, tag="psavt")
                    for it in range(4):
                        nc.tensor.matmul(
                            ps_t[:, it * 128:(it + 1) * 128],
                            attn_sb[:, it * 512 + dt * 128:it * 512 + (dt + 1) * 128],
                            id_sb[:], start=True, stop=True)
                    nc.vector.tensor_copy(avT_sb[:, dt * 512:(dt + 1) * 512], ps_t[:])

                intra_ps = []
                for t in range(4):
                    gl, mt = t // 2, t % 2
                    ps_o = PSI.tile([128, 512], F32, tag=f"psintra{t}")
                    for kt in range(2):
                        blk = gl * 2 + kt
                        nc.tensor.matmul(
                            ps_o[:],
                            Wintra_sb[:, blk * DG + mt * 128:blk * DG + (mt + 1) * 128],
                            avT_sb[:, blk * 512:(blk + 1) * 512],
                            start=(kt == 0), stop=(kt == 1))
                    intra_ps.append(ps_o)
                for mt in range(2):
                    ps_i = PSJ.tile([128, 512], F32, tag="psinter")
                    for kt in range(4):
                        nc.tensor.matmul(
                            ps_i[:],
                            Winter_sb[:, kt * DG + mt * 128:kt * DG + (mt + 1) * 128],
                            avT_sb[:, kt * 512:(kt + 1) * 512],
                            start=(kt == 0), stop=(kt == 3))
                    nc.vector.tensor_copy(inter_sb[:, mt * 512:(mt + 1) * 512], ps_i[:])

                for mt in range(2):
                    nc.sync.dma_start(cc_in[mt * 128:(mt + 1) * 128, :],
                                      inter_sb[:, mt * 512:(mt + 1) * 512])
                nc.gpsimd.collective_compute(
                    "AllReduce", mybir.AluOpType.add,
                    replica_groups=[[0, 1], [2, 3], [4, 5], [6, 7]],
                    ins=[cc_in[:]], outs=[cc_out[:]])
                for mt in range(2):
                    nc.sync.dma_start(inter_rd[:, mt * 512:(mt + 1) * 512],
                                      cc_out[mt * 128:(mt + 1) * 128, :])

                out_f = PE_.tile([128, 4 * 512], F32, tag="outf")
                for t in range(4):
                    mt = t % 2
                    sl = slice(t * 512, (t + 1) * 512)
                    msl = slice(mt * 512, (mt + 1) * 512)
                    tf = W.tile([128, 512], F32, tag="tf")
                    nc.vector.tensor_add(tf[:], intra_ps[t][:], inter_rd[:, msl])
                    nc.vector.tensor_add(out_f[:, sl], tf[:], wres_sb[:, sl])
                    nc.sync.dma_start(out[t * 128:(t + 1) * 128, :], out_f[:, sl])

    nc.finalize()
    return nc


def _host_prep(inputs):
    import concourse.mybir as mybir
    bf = mybir.dt.np(mybir.dt.bfloat16)

    f32 = lambda x: np.ascontiguousarray(np.asarray(x, np.float32))
    tobf = lambda x: np.ascontiguousarray(np.asarray(x, np.float32).astype(bf))

    w = f32(inputs["w"])
    r = f32(inputs["r"])
    mems = f32(inputs["mems"])
    gkv, bkv = f32(inputs["gamma_kv"]), f32(inputs["beta_kv"])
    gq, bq = f32(inputs["gamma_q"]), f32(inputs["beta_q"])
    Wk, Wv = f32(inputs["Wk"]), f32(inputs["Wv"])
    Wq_, Wiq = f32(inputs["Wq"]), f32(inputs["Wiq"])
    Wr_ = f32(inputs["Wr"])
    Wintra, Winter = f32(inputs["Wintra"]), f32(inputs["Winter"])
    rwb_full = f32(inputs["r_w_bias"]).reshape(D)
    rrb_full = f32(inputs["r_r_bias"]).reshape(D)
    kv = np.concatenate([mems, w], 0)
    ident = np.eye(128, dtype=np.float32).astype(bf)

    in_maps = []
    for core in range(8):
        b, s = core // 2, core % 2
        CH0 = 512 * s
        g0, g1 = 2 * s, 2 * s + 1
        perm = np.r_[CH0:CH0 + 512, (512 - CH0):(512 - CH0) + 512]

        qbeta_g = Wiq @ bq
        qbeta = np.concatenate([
            Wq_[g0] @ bq[g0 * DG:(g0 + 1) * DG] + qbeta_g,
            Wq_[g1] @ bq[g1 * DG:(g1 + 1) * DG] + qbeta_g])
        m = {
            "kvT": tobf(kv[:, b, :].T),
            "wTp": tobf(w[:, b, perm].T),
            "wres": f32(w[:, b, CH0:CH0 + 512].T),
            "rT": tobf(r[:, 0, CH0:CH0 + 512].T),
            "WkT": tobf((Wk[CH0:CH0 + 512, :] * gkv[None, :]).T),
            "WvT": tobf((Wv[CH0:CH0 + 512, :] * gkv[None, :]).T),
            "WiqT": tobf((Wiq * gq[None, :]).T[perm, :]),
            "WqT": tobf(np.stack([
                (Wq_[g] * gq[None, g * DG:(g + 1) * DG]).T for g in (g0, g1)])),
            "WrT": tobf(np.stack([Wr_[g].T for g in (g0, g1)])),
            "WintraT": tobf(np.stack([Wintra[g].T for g in (g0, g1)])),
            "WinterT": tobf(Winter[:, CH0:CH0 + 512].T),
            "kbeta": f32(Wk[CH0:CH0 + 512, :] @ bkv).reshape(4, 128).T,
            "rwb": f32(rwb_full[CH0:CH0 + 512] + qbeta).reshape(4, 128).T,
            "rrb": f32(rrb_full[CH0:CH0 + 512] + qbeta).reshape(4, 128).T,
            "ident": ident,
        }
        vbeta = Wv[CH0:CH0 + 512, :] @ bkv
        assert np.abs(vbeta).max() < 1e-6, "nonzero beta_kv for V not supported"
        in_maps.append(m)
    return in_maps


def kernel(**inputs):
    from concourse.bass_utils import run_bass_kernel_spmd

    if "nc" not in _cache:
        _cache["nc"] = _build_nc()
    nc = _cache["nc"]
    in_maps = _host_prep(inputs)
    res = run_bass_kernel_spmd(nc, in_maps, core_ids=list(range(8)))
    _cache["last_results"] = res

    full = np.zeros((Q, B, D), np.float32)
    for core in range(8):
        b, s = core // 2, core % 2
        o = np.asarray(res.results[core]["out"], np.float32)   # [512 ch, 512 q]
        full[:, b, 512 * s:512 * s + 512] = o.T
    return full
